# revision 8
# baseline (speedup 1.0000x reference)
"""Cox partial-likelihood (DeepSurv) loss on 8 TRN2 NeuronCores.

Math: P_exp_sum[i] = sum_j P_exp[j] * (T[i] < T[j]); loss is a scalar
reduction over log(P_exp / (P_exp_sum + eps)) masked by events.

Device does the O(N^2) risk-set sum, data-parallel over rows:
core c owns i in [c*2048, (c+1)*2048). For each 128-wide j-chunk an
engine builds a [128 j, 2048 i] comparison tile and the PE contracts
over j with stationary per-chunk weight columns, accumulating into
PSUM over all 128 chunks:

- 3 of 4 chunks on the DVE: mask = (T_i < T_j) via is_lt with a
  per-partition scalar -> exact {0,1} bf16 (fp32 compare, ties exact).
  Weights are [hi(P_exp_j), lo(P_exp_j)] (bf16 hi/lo split -> ~17-bit
  mantissa).
- 1 of 4 chunks on the ACT engine (load-balancing the mask work):
  smask = Sign(T_j - T_i) in {-1, 0, +1}, consumed by the same matmul
  with weights hi/lo of 0.5*P_exp_j. The sign trick yields
  0.5*(G_i - L_i); the host adds 0.5*S_act and subtracts the exact
  tie-sum 0.5*Eq_i (computed via np.unique; Eq includes j == i) to
  recover G_i = sum over strictly-greater j.

Host does the remaining O(N) epilogue exactly in fp32.
"""

import numpy as np
import ml_dtypes

N = 16384
NCORES = 8
LI = N // NCORES          # rows per core
KC = N // 128             # 128-wide j-chunks
NB = LI // 512            # psum banks per core
EPS = 1e-6

# j-chunks assigned to the ACT engine (Sign path); rest on DVE
ACT_EVERY = 4
ACT_PHASE = 2


def _act_chunks():
    return [k for k in range(KC) if k % ACT_EVERY == ACT_PHASE]


_prog_cache = {}


def _build_program(reps=1):
    if reps in _prog_cache:
        return _prog_cache[reps]
    import concourse.bacc as bacc
    import concourse.tile as tile
    import concourse.mybir as mybir

    act_set = set(_act_chunks())
    nc = bacc.Bacc(
        "TRN2", target_bir_lowering=False, debug=False, num_devices=NCORES
    )
    tib = nc.dram_tensor("tib", [128, LI], mybir.dt.float32, kind="ExternalInput").ap()
    tj = nc.dram_tensor("tj", [128, KC], mybir.dt.float32, kind="ExternalInput").ap()
    w = nc.dram_tensor("w", [128, 2 * KC], mybir.dt.bfloat16, kind="ExternalInput").ap()
    out = nc.dram_tensor("out", [2, LI], mybir.dt.float32, kind="ExternalOutput").ap()

    with tile.TileContext(nc) as tc:
        with (
            tc.tile_pool(name="const", bufs=1) as cpool,
            tc.tile_pool(name="mask", bufs=32) as mpool,
            tc.tile_pool(name="psum", bufs=1, space="PSUM") as ppool,
            tc.tile_pool(name="res", bufs=1) as rpool,
        ):
            tib_s = cpool.tile([128, LI], mybir.dt.float32)
            nc.sync.dma_start(tib_s[:], tib[:])
            tj_s = cpool.tile([128, KC], mybir.dt.float32)
            nc.sync.dma_start(tj_s[:], tj[:])
            w_s = cpool.tile([128, 2 * KC], mybir.dt.bfloat16)
            nc.sync.dma_start(w_s[:], w[:])

            psums = [
                ppool.tile([2, 512], mybir.dt.float32, name=f"psum{b}", tag=f"psum{b}")
                for b in range(NB)
            ]
            res = rpool.tile([2, LI], mybir.dt.float32)
            for _ in range(reps):
                for k in range(KC):
                    mask = mpool.tile(
                        [128, LI], mybir.dt.bfloat16, name="mask", tag="mask"
                    )
                    if k in act_set:
                        nc.scalar.activation(
                            mask[:],
                            tib_s[:],
                            mybir.ActivationFunctionType.Sign,
                            bias=tj_s[:, k : k + 1],
                            scale=-1.0,
                        )
                    else:
                        nc.vector.tensor_scalar(
                            mask[:],
                            tib_s[:],
                            tj_s[:, k : k + 1],
                            None,
                            mybir.AluOpType.is_lt,
                        )
                    for b in range(NB):
                        nc.tensor.matmul(
                            psums[b][:],
                            w_s[:, 2 * k : 2 * k + 2],
                            mask[:, 512 * b : 512 * (b + 1)],
                            start=(k == 0),
                            stop=(k == KC - 1),
                        )
                for b in range(NB):
                    nc.vector.tensor_copy(res[:, 512 * b : 512 * (b + 1)], psums[b][:])
            nc.sync.dma_start(out[:], res[:])
    nc.compile()
    _prog_cache[reps] = nc
    return nc


def _hi_lo(x):
    hi = x.astype(ml_dtypes.bfloat16)
    lo = (x - hi.astype(np.float32)).astype(ml_dtypes.bfloat16)
    return hi, lo


def _make_in_maps(P_risk, T):
    P_exp = np.exp(P_risk.astype(np.float32))
    # DVE chunks: weights = (hi, lo) of P_exp; ACT chunks: of 0.5*P_exp
    # (the sign mask contributes G - L; the 0.5 folds the averaging in).
    wfull = P_exp.copy()
    act_j = np.zeros(N, dtype=bool)
    for k in _act_chunks():
        act_j[k * 128 : (k + 1) * 128] = True
    wfull[act_j] *= np.float32(0.5)
    hi, lo = _hi_lo(wfull)
    # w[p, 2k+0] = hi[k*128+p], w[p, 2k+1] = lo[k*128+p]
    w = np.empty((128, 2 * KC), dtype=ml_dtypes.bfloat16)
    w[:, 0::2] = hi.reshape(KC, 128).T
    w[:, 1::2] = lo.reshape(KC, 128).T
    tjv = np.ascontiguousarray(T.astype(np.float32).reshape(KC, 128).T)
    in_maps = []
    for c in range(NCORES):
        tib = np.ascontiguousarray(
            np.broadcast_to(T[c * LI : (c + 1) * LI].astype(np.float32), (128, LI))
        )
        in_maps.append({"tib": tib, "tj": tjv, "w": w})
    return in_maps, P_exp


def _sign_correction(P_exp, T):
    """Per-row correction recovering G from the ACT chunks' 0.5*(G-L):
    add 0.5*S_act - 0.5*Eq_i, with Eq_i the exact sum of P_exp over
    ACT-chunk j with T_j == T_i (self included)."""
    act_j = np.zeros(N, dtype=bool)
    for k in _act_chunks():
        act_j[k * 128 : (k + 1) * 128] = True
    S_act = np.float32(P_exp[act_j].sum(dtype=np.float64))
    uniq, inv = np.unique(T, return_inverse=True)
    eq_group = np.zeros(len(uniq), np.float32)
    np.add.at(eq_group, inv[act_j], P_exp[act_j])
    Eq = eq_group[inv]
    return np.float32(0.5) * S_act - np.float32(0.5) * Eq


def _epilogue(P_risk, T, E, P_exp, P_exp_sum):
    T = T.astype(np.float32)
    has_risk = (T < T.max()).astype(np.float32)
    Ef = E.astype(np.float32) * has_risk
    P_tmp = P_exp / (P_exp_sum + np.float32(EPS))
    upper = P_tmp.max()
    P_clipped = np.clip(P_tmp, np.float32(EPS), upper)
    loss = -np.sum(np.log(P_clipped) * Ef, dtype=np.float32) / np.sum(
        Ef, dtype=np.float32
    )
    return np.asarray(loss, dtype=np.float32)


def kernel(P_risk, T, E):
    from concourse.bass_utils import run_bass_kernel_spmd

    nc = _build_program()
    in_maps, P_exp = _make_in_maps(P_risk, T)
    corr = _sign_correction(P_exp, T.astype(np.float32))
    S_total = float(P_exp.sum(dtype=np.float64))
    last_err = None
    for _attempt in range(3):
        try:
            res = run_bass_kernel_spmd(nc, in_maps, core_ids=list(range(NCORES)))
            outs = np.stack([res.results[c]["out"] for c in range(NCORES)])
            g = (outs[:, 0, :] + outs[:, 1, :]).reshape(N)
            P_exp_sum = g + corr
            # sanity: each risk-set sum lies in [0, sum(P_exp)]; the row
            # holding max(T) has an empty risk set. Guards against a
            # silently-failed device execution.
            ok = (
                np.isfinite(P_exp_sum).all()
                and float(P_exp_sum.min()) >= -1e-2
                and float(P_exp_sum.max()) <= S_total * 1.001
                and abs(float(P_exp_sum[int(np.argmax(T))])) < 1e-2
                and float(P_exp_sum.max()) > 0.0
            )
            if ok:
                return _epilogue(P_risk, T, E, P_exp, P_exp_sum)
            last_err = RuntimeError("device output failed sanity check")
        except Exception as e:  # transient NRT device errors happen
            last_err = e
    raise last_err
